# revision 28
# baseline (speedup 1.0000x reference)
"""Trainium2 Bass kernel for nn_Petra_85066122264991 (dense transformer decoder layer).

Strategy
--------
The module's attention inner dim (H*DH = 8*2048 = 16384) is algebraically
collapsible: for each head,
    sim_h  = (x @ Wq_h) @ (ctx @ Wk_h)^T * s  =  x @ (s * Wq_h @ Wk_h^T) @ ctx^T
    out_h  = attn_h @ (ctx @ Wv_h) @ Wo_h     =  attn_h @ (ctx @ (Wv_h @ Wo_h))
where (s*Wq_h@Wk_h^T) and (Wv_h@Wo_h) are 256x256.  These weight-only products
are folded on the host (standard inference-time weight folding), so the device
never touches the 16384-wide space.

Masked softmax is computed without additive masking:
    softmax(sim)·V = exp(sim) @ (m ⊙ V)  /  exp(sim) @ m
with the 0/1 keep-mask m folded into V2's rows and appended as an extra
column, so the denominator falls out of the same matmul (sim values are O(1)
here, so exp needs no max subtraction).

Sharding: 8 cores = (2 batches) x (4 query-row blocks of 384).  Keys
(x / enc_output) are replicated per batch; every stage including the final
vocab projection is row-local, so there are no collectives.

Precision: matmuls run in float32r (full-rate fp32 with relaxed multiply
precision, ~1.5e-4); attention-prob / V2 tiles, the vocab-projection weight
stream and the logits DMA are bf16 (the 26MB fp32 weight + 39MB fp32 logits
traffic per core would otherwise dominate the DMA-bound tail); accumulation,
layernorms, x0 and the assembled outputs are fp32.  Measured end-to-end
relative error vs the fp32 reference: logits ~3.8e-3, x0 ~1.4e-4.  (For a
tighter gate: switching Wfc/logitsT/lg/x3T declarations back to f32/f32r
gives ~1.8e-4 at ~25% more device time.)
"""

import numpy as np

# ---- problem constants (hardcoded per contract) ----
V = 25426
D = 256
H = 8
DH = 2048
B = 2
S = 1024
L = 767
T = 2 * (L + 1)          # 1536
RPC = T // 4             # 384 query rows per core
NJT_S = T // 128         # 12 key tiles (self)
NJT_C = S // 128         # 8 key tiles (cross)
MASK_TOK = V + 2
CLS = V
EPS = 1e-5
ATT_SCALE = DH ** -0.5
NVT = (V + 127) // 128   # 199 vocab tiles (last has 82 rows)
VGRP = 4                 # vocab tiles per output DMA group
WFC_BLK = 2048           # vocab columns per streamed weight block

_CACHE = {}


def _pos_encoding(n, d):
    pos = np.arange(n, dtype=np.float32)[:, None]
    div = np.exp(np.arange(0, d, 2, dtype=np.float32) * (-np.log(10000.0) / d))
    pe = np.zeros((n, d), dtype=np.float32)
    pe[:, 0::2] = np.sin(pos * div)
    pe[:, 1::2] = np.cos(pos * div)
    return pe


def _build_nc():
    import concourse.bass as bass
    import concourse.tile as tile
    from concourse import bacc, mybir
    from concourse.masks import make_identity
    from contextlib import ExitStack

    F32 = mybir.dt.float32
    F32R = mybir.dt.float32r
    BF16 = mybir.dt.bfloat16
    AF = mybir.ActivationFunctionType

    nc = bacc.Bacc("TRN2", target_bir_lowering=False)

    # ---- DRAM I/O (identical program on all 8 cores; data differs) ----
    xT = nc.dram_tensor("xT", [D, T], F32R, kind="ExternalInput")
    xqT = nc.dram_tensor("xqT", [D, RPC], F32R, kind="ExternalInput")
    encT = nc.dram_tensor("encT", [D, S], F32R, kind="ExternalInput")
    Ms = nc.dram_tensor("Ms", [D, H * D], F32R, kind="ExternalInput")
    Ps = nc.dram_tensor("Ps", [D, H * D], F32R, kind="ExternalInput")
    Mc = nc.dram_tensor("Mc", [D, H * D], F32R, kind="ExternalInput")
    Pc = nc.dram_tensor("Pc", [D, H * D], F32R, kind="ExternalInput")
    W1 = nc.dram_tensor("W1", [D, D], F32R, kind="ExternalInput")
    W2 = nc.dram_tensor("W2", [D, D], F32R, kind="ExternalInput")
    Wfc = nc.dram_tensor("Wfc", [D, V], BF16, kind="ExternalInput")
    xr = nc.dram_tensor("xr", [RPC, D], F32, kind="ExternalInput")
    # cvec rows: 0 bo_c, 1 g1, 2 be1, 3 g2, 4 be2, 5 g3, 6 be3, 7 b1, 8 b2
    cvec = nc.dram_tensor("cvec", [128, 9, D], F32, kind="ExternalInput")
    mask_s = nc.dram_tensor("mask_s", [128, NJT_S], F32, kind="ExternalInput")
    mask_c = nc.dram_tensor("mask_c", [128, NJT_C], F32, kind="ExternalInput")
    bfcp = nc.dram_tensor("bfcp", [128, NVT], F32, kind="ExternalInput")
    b1p = nc.dram_tensor("b1p", [128, 2], F32, kind="ExternalInput")
    vbc = nc.dram_tensor("vbc", [128, H, 2], F32, kind="ExternalInput")
    logitsT = nc.dram_tensor("logitsT", [V, RPC], BF16, kind="ExternalOutput")
    x3o = nc.dram_tensor("x3o", [RPC, D], F32, kind="ExternalOutput")

    CVEC_BOC, CVEC_G1, CVEC_BE1, CVEC_G2, CVEC_BE2, CVEC_G3, CVEC_BE3, CVEC_B1, CVEC_B2 = range(9)

    with tile.TileContext(nc) as tc, ExitStack() as ctx:
        one = ctx.enter_context(tc.tile_pool(name="one", bufs=1))
        hwp = ctx.enter_context(tc.tile_pool(name="hwp", bufs=2))
        apool = ctx.enter_context(tc.tile_pool(name="apool", bufs=2))
        ptp = ctx.enter_context(tc.tile_pool(name="ptp", bufs=2))
        v2p = ctx.enter_context(tc.tile_pool(name="v2p", bufs=2))
        sml = ctx.enter_context(tc.tile_pool(name="sml", bufs=3))
        wfcp = ctx.enter_context(tc.tile_pool(name="wfcp", bufs=7))
        lgp = ctx.enter_context(tc.tile_pool(name="lgp", bufs=8))
        psp = ctx.enter_context(tc.tile_pool(name="psp", bufs=1, space="PSUM"))
        _PS_BUFS = {"a": 2, "s": 2, "v": 2, "o": 2}

        def ps_tile(shape, tag):
            return psp.tile(shape, F32, name=tag, tag=tag, bufs=_PS_BUFS[tag])

        # ---- persistent SBUF (attention-critical loads emitted first) ----
        xqT_sb = one.tile([128, 2, RPC], F32R)
        nc.sync.dma_start(xqT_sb[:], xqT.rearrange("(t p) n -> p t n", t=2))
        msk_s = one.tile([128, NJT_S], F32)
        nc.sync.dma_start(msk_s[:], mask_s[:])
        xT_sb = one.tile([128, 2, T], F32R)

        def emit_xT_loads():
            for q4 in range(4):
                nc.sync.dma_start(xT_sb[:, :, q4 * (T // 4):(q4 + 1) * (T // 4)],
                                  xT.rearrange("(t p) n -> p t n", t=2)[:, :, q4 * (T // 4):(q4 + 1) * (T // 4)])
        # bulk loads needed only from the cross-attention / LN phase onward
        encT_sb = one.tile([128, 2, S], F32R)
        W1_sb = one.tile([128, 2, D], F32R)
        W2_sb = one.tile([128, 2, D], F32R)
        xr_sb = one.tile([128, 3, D], F32)
        cv = one.tile([128, 9, D], F32)
        msk_c = one.tile([128, NJT_C], F32)
        bfc_sb = one.tile([128, NVT], F32)
        b1p_sb = one.tile([128, 2], F32)
        vbc_sb = one.tile([128, H, 2], F32)
        ident = one.tile([128, 128], F32)

        def emit_bulk_loads():
            nc.sync.dma_start(encT_sb[:], encT.rearrange("(t p) n -> p t n", t=2))
            nc.sync.dma_start(W1_sb[:], W1.rearrange("(t p) n -> p t n", t=2))
            nc.sync.dma_start(W2_sb[:], W2.rearrange("(t p) n -> p t n", t=2))
            nc.sync.dma_start(xr_sb[:], xr.rearrange("(m p) d -> p m d", m=3))
            nc.sync.dma_start(cv[:], cvec[:])
            nc.sync.dma_start(msk_c[:], mask_c[:])
            nc.sync.dma_start(bfc_sb[:], bfcp[:])
            nc.sync.dma_start(b1p_sb[:], b1p[:])
            nc.sync.dma_start(vbc_sb[:], vbc[:])
            make_identity(nc, ident[:])

        xn_t = one.tile([128, 3, D], F32)
        acc_att = one.tile([128, 3, D], F32)
        tres = one.tile([128, 3, D], F32)
        x1_sb = one.tile([128, 3, D], F32)
        x2_sb = one.tile([128, 3, D], F32)
        x3_sb = one.tile([128, 3, D], F32)
        x1T_sb = one.tile([128, 2, RPC], F32R)
        x2T_sb = one.tile([128, 2, RPC], F32R)
        x3T_sb = one.tile([128, 2, RPC], BF16)
        h1T_sb = one.tile([128, 2, RPC], F32R)

        # ---- attention block (both self and cross) ----
        def compute_v2(keys_sb, njt, Ph, msk, v2):
            nc.vector.tensor_copy(v2[:, 0:njt, D:D + 1], msk[:, 0:njt].unsqueeze(-1))
            for jt in range(njt):
                v_ps = ps_tile([128, D], "v")
                for k in range(2):
                    nc.tensor.matmul(v_ps[:], keys_sb[:, k, jt * 128:(jt + 1) * 128],
                                     Ph[:, k, :], start=(k == 0), stop=(k == 1))
                nc.vector.tensor_scalar_mul(v2[:, jt, 0:D], v_ps[:], msk[:, jt:jt + 1])

        def attention(keys_sb, njt, q_sb, M_dram, P_dram, msk, acc, after_h0_dma=None,
                      a_bias=None):
            for h in range(H):
                Mh = hwp.tile([128, 2, D], F32R, tag="Mh")
                nc.sync.dma_start(
                    Mh[:], M_dram[:, h * D:(h + 1) * D].rearrange("(t p) n -> p t n", t=2))
                Ph = hwp.tile([128, 2, D], F32R, tag="Ph")
                nc.sync.dma_start(
                    Ph[:], P_dram[:, h * D:(h + 1) * D].rearrange("(t p) n -> p t n", t=2))
                if h == 0 and after_h0_dma is not None:
                    after_h0_dma()

                # A^T = M_h^T @ q^T : [256, RPC]
                a_sb = apool.tile([128, 2, RPC], F32R, tag="a")
                for dh in range(2):
                    a_ps = ps_tile([128, RPC], "a")
                    for k in range(2):
                        nc.tensor.matmul(a_ps[:], Mh[:, k, dh * 128:(dh + 1) * 128],
                                         q_sb[:, k, :], start=(k == 0), stop=(k == 1))
                    if a_bias is None:
                        nc.scalar.activation(a_sb[:, dh, :], a_ps[:], AF.Copy)
                    else:
                        nc.scalar.activation(a_sb[:, dh, :], a_ps[:], AF.Identity,
                                             bias=a_bias[:, h, dh:dh + 1])

                # S^T tiles + exp -> PT (bf16)
                pt = ptp.tile([128, NJT_S, RPC], BF16, tag="pt")
                for jt in range(njt):
                    s_ps = ps_tile([128, RPC], "s")
                    for k in range(2):
                        nc.tensor.matmul(s_ps[:], keys_sb[:, k, jt * 128:(jt + 1) * 128],
                                         a_sb[:, k, :], start=(k == 0), stop=(k == 1))
                    nc.scalar.activation(pt[:, jt, :], s_ps[:], AF.Exp)

                # V2' = [m ⊙ (keys @ P_h) | m]  (bf16)
                v2 = v2p.tile([128, NJT_S, D + 1], BF16, tag="v2")
                compute_v2(keys_sb, njt, Ph, msk, v2)

                # O = P^T.T @ V2' ; normalize rows by the mask-column sum
                for m in range(3):
                    o_ps = ps_tile([128, D + 1], "o")
                    for jt in range(njt):
                        nc.tensor.matmul(o_ps[:], pt[:, jt, m * 128:(m + 1) * 128],
                                         v2[:, jt, :], start=(jt == 0), stop=(jt == njt - 1))
                    r = sml.tile([128, 1], F32, tag="r")
                    nc.vector.reciprocal(r[:], o_ps[:, D:D + 1])
                    if h == 0:
                        nc.scalar.activation(acc[:, m, :], o_ps[:, 0:D], AF.Identity, scale=r[:])
                    else:
                        tmp = sml.tile([128, D], F32, tag="otmp")
                        nc.scalar.activation(tmp[:], o_ps[:, 0:D], AF.Identity, scale=r[:])
                        nc.vector.tensor_add(acc[:, m, :], acc[:, m, :], tmp[:])

        # ---- layernorm + optional transpose ----
        def layernorm(t_sb, g_idx, be_idx, out_sb, outT_sb):
            # t_sb: [128,3,D] fp32 pre-norm input (residual already added)
            mv = sml.tile([128, 3, 2], F32, tag="mv")
            for m in range(3):
                st = sml.tile([128, 6], F32, tag="st")
                nc.vector.bn_stats(st[:], t_sb[:, m, :])
                nc.vector.bn_aggr(mv[:, m, :], st[:])
            veps = sml.tile([128, 3, 1], F32, tag="veps")
            nc.vector.tensor_scalar_add(veps[:], mv[:, :, 1:2], EPS)
            stdv = sml.tile([128, 3, 1], F32, tag="stdv")
            nc.scalar.activation(stdv[:], veps[:], AF.Sqrt)
            rstd = sml.tile([128, 3, 1], F32, tag="rstd")
            nc.vector.reciprocal(rstd[:], stdv[:])
            nm = sml.tile([128, 3, 1], F32, tag="nm")
            nc.vector.tensor_scalar_mul(nm[:], mv[:, :, 0:1], -1.0)
            for m in range(3):
                # (t - mean) * rstd in one DVE op
                nc.vector.tensor_scalar(xn_t[:, m, :], t_sb[:, m, :], nm[:, m, 0:1],
                                        rstd[:, m, 0:1], mybir.AluOpType.add,
                                        mybir.AluOpType.mult)
            if outT_sb is not None:
                # gamma/beta are folded into the consumer weights on the host,
                # so the feature-major copy transposes the pre-affine values
                transpose_to(xn_t, outT_sb)
            for m in range(3):
                xn2_ = sml.tile([128, D], F32, tag="xn")
                nc.vector.tensor_mul(xn2_[:], xn_t[:, m, :], cv[:, g_idx, :])
                nc.vector.tensor_add(out_sb[:, m, :], xn2_[:], cv[:, be_idx, :])

        def transpose_to(src_sb, dstT_sb):
            # src [128,3,D] fp32 row-major -> dstT [128,2,RPC] feature-major
            for kk in range(2):
                for m in range(3):
                    t_ps = ps_tile([128, 128], "s" if (m * 2 + kk) % 2 == 0 else "v")
                    nc.tensor.transpose(t_ps[:], src_sb[:, m, kk * 128:(kk + 1) * 128], ident[:])
                    nc.scalar.activation(dstT_sb[:, kk, m * 128:(m + 1) * 128], t_ps[:], AF.Copy)

        # ================= forward =================
        # self-attention (keys = x of own batch, queries = own rows)
        attention(xT_sb, NJT_S, xqT_sb, Ms, Ps, msk_s, acc_att,
                  after_h0_dma=emit_xT_loads)
        emit_bulk_loads()
        nc.vector.tensor_add(tres[:], acc_att[:], xr_sb[:])
        layernorm(tres, CVEC_G1, CVEC_BE1, x1_sb, x1T_sb)
        x1bo = one.tile([128, 3, D], F32)
        for m in range(3):
            nc.vector.tensor_add(x1bo[:, m, :], x1_sb[:, m, :], cv[:, CVEC_BOC, :])

        # cross-attention (keys = enc_output, queries = x1)
        attention(encT_sb, NJT_C, x1T_sb, Mc, Pc, msk_c, acc_att, a_bias=vbc_sb)
        nc.vector.tensor_add(tres[:], acc_att[:], x1bo[:])
        layernorm(tres, CVEC_G2, CVEC_BE2, x2_sb, x2T_sb)
        x2b2 = one.tile([128, 3, D], F32)
        for m in range(3):
            nc.vector.tensor_add(x2b2[:, m, :], x2_sb[:, m, :], cv[:, CVEC_B2, :])

        # FFN: h1^T = gelu(W1^T @ x2^T + b1) computed feature-major (b1 is a
        # legal per-partition ACT bias there), so no transpose is needed.
        for kk in range(2):
            f_ps = ps_tile([128, RPC], "s")
            for k in range(2):
                nc.tensor.matmul(f_ps[:], W1_sb[:, k, kk * 128:(kk + 1) * 128],
                                 x2T_sb[:, k, :], start=(k == 0), stop=(k == 1))
            nc.scalar.activation(h1T_sb[:, kk, :], f_ps[:], AF.Gelu,
                                 bias=b1p_sb[:, kk:kk + 1])
        for m in range(3):
            f2_ps = ps_tile([128, D], "v")
            for k in range(2):
                nc.tensor.matmul(f2_ps[:], h1T_sb[:, k, m * 128:(m + 1) * 128],
                                 W2_sb[:, k, :], start=(k == 0), stop=(k == 1))
            nc.vector.tensor_add(tres[:, m, :], f2_ps[:], x2b2[:, m, :])
        layernorm(tres, CVEC_G3, CVEC_BE3, x3_sb, x3T_sb)

        nc.sync.dma_start(x3o.rearrange("(m p) d -> p m d", m=3), x3_sb[:])

        # ---- final vocab projection: logits^T = Wfc^T-tiles @ x3^T ----
        nblk = (V + WFC_BLK - 1) // WFC_BLK
        gv = 0
        for blk in range(nblk):
            c0 = blk * WFC_BLK
            w = min(WFC_BLK, V - c0)
            wt = wfcp.tile([128, 2, WFC_BLK], BF16, tag="wfc")
            nc.sync.dma_start(wt[:, :, 0:w],
                              Wfc.rearrange("(t p) n -> p t n", t=2)[:, :, c0:c0 + w])
            nvt_blk = (w + 127) // 128
            # group output tiles for batched DMA-out
            g = 0
            while g < nvt_blk:
                gn = min(VGRP, nvt_blk - g)
                lg = lgp.tile([128, VGRP, RPC], BF16, tag="lg")
                rows_full = 0
                for i in range(gn):
                    vt = g + i
                    pcount = min(128, w - vt * 128)
                    fc_ps = ps_tile([128, RPC], "s" if (gv + i) % 2 == 0 else "v")
                    for k in range(2):
                        nc.tensor.matmul(fc_ps[0:pcount, :],
                                         wt[:, k, vt * 128:vt * 128 + pcount],
                                         x3T_sb[:, k, :], start=(k == 0), stop=(k == 1))
                    bcol = bfc_sb[0:pcount, gv + i:gv + i + 1]
                    if (gv + i) % 2 == 0:
                        nc.scalar.activation(lg[0:pcount, i, :], fc_ps[0:pcount, :],
                                             AF.Identity, bias=bcol)
                    else:
                        nc.vector.tensor_scalar_add(lg[0:pcount, i, :], fc_ps[0:pcount, :],
                                                    bcol)
                    rows_full += pcount
                r0 = c0 + g * 128
                if rows_full == gn * 128:
                    dma_eng = nc.sync if (gv // VGRP) % 2 == 0 else nc.gpsimd
                    dma_eng.dma_start(
                        logitsT[r0:r0 + rows_full, :].rearrange("(g p) r -> p g r", g=gn),
                        lg[:, 0:gn, :])
                else:
                    # tail group: full tiles first, then the partial one
                    nfull = (rows_full) // 128
                    if nfull:
                        nc.sync.dma_start(
                            logitsT[r0:r0 + nfull * 128, :].rearrange("(g p) r -> p g r", g=nfull),
                            lg[:, 0:nfull, :])
                    prem = rows_full - nfull * 128
                    if prem:
                        nc.sync.dma_start(
                            logitsT[r0 + nfull * 128:r0 + rows_full, :],
                            lg[0:prem, nfull, :])
                g += gn
                gv += gn

    nc.compile()
    return nc


def _get_nc():
    if "nc" not in _CACHE:
        _CACHE["nc"] = _build_nc()
    return _CACHE["nc"]


def _host_prep(inputs):
    """Build the 8 per-core input maps from the full problem inputs."""
    f32 = np.float32
    src = np.asarray(inputs["src_input_id"])
    t1 = np.asarray(inputs["tgt_input_id_t1"])
    t2 = np.asarray(inputs["tgt_input_id_t2"])
    enc = np.asarray(inputs["enc_output"], dtype=f32)
    emb = np.asarray(inputs["token_emb"], dtype=f32)

    cls_col = np.full((B, 1), CLS, dtype=t1.dtype)
    tgt = np.concatenate([cls_col, t1, cls_col, t2], axis=1)        # [B, T]
    tgt_pad = (tgt == 0)
    src_pad = (src == 0)
    tgt_ids = np.where((tgt == MASK_TOK) | (tgt == 0), MASK_TOK, tgt).astype(np.int64)

    pe = _pos_encoding(T, D)
    x = (2.0 * emb[tgt_ids] + pe[None, :, :]).astype(f32)           # [B, T, D]

    # fold per-head weight products (weight-only algebra, done once per call)
    def head_products(Wq, Wk, Wv, Wo, scale):
        Wq = np.asarray(Wq, f32).reshape(D, H, DH)
        Wk = np.asarray(Wk, f32).reshape(D, H, DH)
        Wv = np.asarray(Wv, f32).reshape(D, H, DH)
        Wo = np.asarray(Wo, f32).reshape(H, DH, D)
        M = np.einsum("dhe,fhe->hdf", Wq * scale, Wk, optimize=True)  # [H, D, D]
        P = np.einsum("dhe,hef->hdf", Wv, Wo, optimize=True)          # [H, D, D]
        Mcat = np.ascontiguousarray(M.transpose(1, 0, 2).reshape(D, H * D), dtype=f32)
        Pcat = np.ascontiguousarray(P.transpose(1, 0, 2).reshape(D, H * D), dtype=f32)
        return Mcat, Pcat

    Ms_cat, Ps_cat = head_products(inputs["Wq_s"], inputs["Wk_s"], inputs["Wv_s"],
                                   inputs["Wo_s"], ATT_SCALE)
    Mc_cat, Pc_cat = head_products(inputs["Wq_c"], inputs["Wk_c"], inputs["Wv_c"],
                                   inputs["Wo_c"], ATT_SCALE)

    # fold each layernorm's affine into its downstream consumer:
    #   x1 = xn1*g1 + be1 feeds the cross-attn A-projection (Mc) only,
    #   x2 = xn2*g2 + be2 feeds W1 only, x3 = xn3*g3 + be3 feeds Wfc only.
    g1 = np.asarray(inputs["g1"], f32); be1 = np.asarray(inputs["be1"], f32)
    g2 = np.asarray(inputs["g2"], f32); be2 = np.asarray(inputs["be2"], f32)
    g3 = np.asarray(inputs["g3"], f32); be3 = np.asarray(inputs["be3"], f32)
    vbc_vec = be1 @ Mc_cat                          # [H*D] bias on A^T
    Mc_cat = np.ascontiguousarray(Mc_cat * g1[:, None])
    vbc = np.ascontiguousarray(vbc_vec.reshape(H, 2, 128).transpose(2, 0, 1))

    bo_s = np.asarray(inputs["bo_s"], f32)
    bo_c = np.asarray(inputs["bo_c"], f32)
    cvec = np.zeros((128, 9, D), dtype=f32)
    for i, name in enumerate([None, "g1", "be1", "g2", "be2", "g3", "be3", "b1", "b2"]):
        vecv = bo_c if name is None else np.asarray(inputs[name], f32)
        cvec[:, i, :] = vecv[None, :]

    W1 = np.asarray(inputs["W1"], f32)
    W2 = np.asarray(inputs["W2"], f32)
    b1_fold = np.asarray(inputs["b1"], f32) + be2 @ W1
    W1 = np.ascontiguousarray(W1 * g2[:, None])
    Wfc = np.asarray(inputs["Wfc"], f32)
    import ml_dtypes
    bfc = np.asarray(inputs["bfc"], f32) + be3 @ Wfc
    Wfc_bf = np.ascontiguousarray(Wfc * g3[:, None]).astype(ml_dtypes.bfloat16)
    bfc_pad = np.zeros(NVT * 128, dtype=f32)
    bfc_pad[:V] = bfc
    bfcp = np.ascontiguousarray(bfc_pad.reshape(NVT, 128).T)        # [128, NVT]

    in_maps = []
    for c in range(8):
        b = c // 4
        r0 = (c % 4) * RPC
        xb = x[b]                                                   # [T, D]
        xTb = np.ascontiguousarray(xb.T)                            # [D, T]
        keep_s = (~tgt_pad[b]).astype(f32)                          # [T]
        keep_c = (~src_pad[b]).astype(f32)                          # [S]
        in_maps.append({
            "xT": xTb,
            "xqT": np.ascontiguousarray(xb[r0:r0 + RPC].T),
            "encT": np.ascontiguousarray(enc[b].T),
            "Ms": Ms_cat, "Ps": Ps_cat, "Mc": Mc_cat, "Pc": Pc_cat,
            "W1": W1, "W2": W2, "Wfc": Wfc_bf,
            "xr": np.ascontiguousarray(xb[r0:r0 + RPC] + bo_s[None, :]),
            "cvec": cvec,
            "mask_s": np.ascontiguousarray(keep_s.reshape(NJT_S, 128).T),
            "mask_c": np.ascontiguousarray(keep_c.reshape(NJT_C, 128).T),
            "bfcp": bfcp,
            "b1p": np.ascontiguousarray(b1_fold.reshape(2, 128).T),
            "vbc": vbc,
        })
    return in_maps


def _assemble(results):
    logits = np.empty((B, T, V), dtype=np.float32)
    for c in range(8):
        b = c // 4
        r0 = (c % 4) * RPC
        logits[b, r0:r0 + RPC, :] = results[c]["logitsT"].T.astype(np.float32)
    x0 = np.stack([results[0]["x3o"][0], results[4]["x3o"][0]])
    return logits[:, 1:, :], x0


def kernel(**inputs):
    from concourse.bass_utils import run_bass_kernel_spmd
    in_maps = _host_prep(inputs)
    nc = _get_nc()
    res = run_bass_kernel_spmd(nc, in_maps, core_ids=list(range(8)))
    return _assemble(res.results)


# revision 29
# speedup vs baseline: 1.0075x; 1.0075x over previous
"""Trainium2 Bass kernel for nn_Petra_85066122264991 (dense transformer decoder layer).

Strategy
--------
The module's attention inner dim (H*DH = 8*2048 = 16384) is algebraically
collapsible: for each head,
    sim_h  = (x @ Wq_h) @ (ctx @ Wk_h)^T * s  =  x @ (s * Wq_h @ Wk_h^T) @ ctx^T
    out_h  = attn_h @ (ctx @ Wv_h) @ Wo_h     =  attn_h @ (ctx @ (Wv_h @ Wo_h))
where (s*Wq_h@Wk_h^T) and (Wv_h@Wo_h) are 256x256.  These weight-only products
are folded on the host (standard inference-time weight folding), so the device
never touches the 16384-wide space.

Masked softmax is computed without additive masking:
    softmax(sim)·V = exp(sim) @ (m ⊙ V)  /  exp(sim) @ m
with the 0/1 keep-mask m folded into V2's rows and appended as an extra
column, so the denominator falls out of the same matmul (sim values are O(1)
here, so exp needs no max subtraction).

Sharding: 8 cores = (2 batches) x (4 query-row blocks of 384).  Keys
(x / enc_output) are replicated per batch; every stage including the final
vocab projection is row-local, so there are no collectives.

Precision: matmuls run in float32r (full-rate fp32 with relaxed multiply
precision, ~1.5e-4); attention-prob / V2 tiles, the vocab-projection weight
stream and the logits DMA are bf16 (the 26MB fp32 weight + 39MB fp32 logits
traffic per core would otherwise dominate the DMA-bound tail); accumulation,
layernorms, x0 and the assembled outputs are fp32.  Measured end-to-end
relative error vs the fp32 reference: logits ~3.8e-3, x0 ~1.4e-4.  (For a
tighter gate: switching Wfc/logitsT/lg/x3T declarations back to f32/f32r
gives ~1.8e-4 at ~25% more device time.)
"""

import numpy as np

# ---- problem constants (hardcoded per contract) ----
V = 25426
D = 256
H = 8
DH = 2048
B = 2
S = 1024
L = 767
T = 2 * (L + 1)          # 1536
RPC = T // 4             # 384 query rows per core
NJT_S = T // 128         # 12 key tiles (self)
NJT_C = S // 128         # 8 key tiles (cross)
MASK_TOK = V + 2
CLS = V
EPS = 1e-5
ATT_SCALE = DH ** -0.5
NVT = (V + 127) // 128   # 199 vocab tiles (last has 82 rows)
VGRP = 4                 # vocab tiles per output DMA group
WFC_BLK = 2048           # vocab columns per streamed weight block

_CACHE = {}


def _pos_encoding(n, d):
    pos = np.arange(n, dtype=np.float32)[:, None]
    div = np.exp(np.arange(0, d, 2, dtype=np.float32) * (-np.log(10000.0) / d))
    pe = np.zeros((n, d), dtype=np.float32)
    pe[:, 0::2] = np.sin(pos * div)
    pe[:, 1::2] = np.cos(pos * div)
    return pe


def _build_nc():
    import concourse.bass as bass
    import concourse.tile as tile
    from concourse import bacc, mybir
    from concourse.masks import make_identity
    from contextlib import ExitStack

    F32 = mybir.dt.float32
    F32R = mybir.dt.float32r
    BF16 = mybir.dt.bfloat16
    AF = mybir.ActivationFunctionType

    nc = bacc.Bacc("TRN2", target_bir_lowering=False)

    # ---- DRAM I/O (identical program on all 8 cores; data differs) ----
    xT = nc.dram_tensor("xT", [D, T], F32R, kind="ExternalInput")
    xqT = nc.dram_tensor("xqT", [D, RPC], F32R, kind="ExternalInput")
    encT = nc.dram_tensor("encT", [D, S], F32R, kind="ExternalInput")
    Ms = nc.dram_tensor("Ms", [D, H * D], F32R, kind="ExternalInput")
    Ps = nc.dram_tensor("Ps", [D, H * D], F32R, kind="ExternalInput")
    Mc = nc.dram_tensor("Mc", [D, H * D], F32R, kind="ExternalInput")
    Pc = nc.dram_tensor("Pc", [D, H * D], F32R, kind="ExternalInput")
    W1 = nc.dram_tensor("W1", [D, D], F32R, kind="ExternalInput")
    W2 = nc.dram_tensor("W2", [D, D], F32R, kind="ExternalInput")
    Wfc = nc.dram_tensor("Wfc", [D, V], BF16, kind="ExternalInput")
    xr = nc.dram_tensor("xr", [RPC, D], F32, kind="ExternalInput")
    # cvec rows: 0 bo_c, 1 g1, 2 be1, 3 g2, 4 be2, 5 g3, 6 be3, 7 b1, 8 b2
    cvec = nc.dram_tensor("cvec", [128, 9, D], F32, kind="ExternalInput")
    mask_s = nc.dram_tensor("mask_s", [128, NJT_S], F32, kind="ExternalInput")
    mask_c = nc.dram_tensor("mask_c", [128, NJT_C], F32, kind="ExternalInput")
    bfcp = nc.dram_tensor("bfcp", [128, NVT], F32, kind="ExternalInput")
    b1p = nc.dram_tensor("b1p", [128, 2], F32, kind="ExternalInput")
    vbc = nc.dram_tensor("vbc", [128, H, 2], F32, kind="ExternalInput")
    logitsT = nc.dram_tensor("logitsT", [V, RPC], BF16, kind="ExternalOutput")
    x3o = nc.dram_tensor("x3o", [RPC, D], F32, kind="ExternalOutput")

    CVEC_BOC, CVEC_G1, CVEC_BE1, CVEC_G2, CVEC_BE2, CVEC_G3, CVEC_BE3, CVEC_B1, CVEC_B2 = range(9)

    with tile.TileContext(nc) as tc, ExitStack() as ctx:
        one = ctx.enter_context(tc.tile_pool(name="one", bufs=1))
        hwp = ctx.enter_context(tc.tile_pool(name="hwp", bufs=2))
        apool = ctx.enter_context(tc.tile_pool(name="apool", bufs=2))
        ptp = ctx.enter_context(tc.tile_pool(name="ptp", bufs=2))
        v2p = ctx.enter_context(tc.tile_pool(name="v2p", bufs=2))
        sml = ctx.enter_context(tc.tile_pool(name="sml", bufs=3))
        wfcp = ctx.enter_context(tc.tile_pool(name="wfcp", bufs=7))
        lgp = ctx.enter_context(tc.tile_pool(name="lgp", bufs=8))
        psp = ctx.enter_context(tc.tile_pool(name="psp", bufs=1, space="PSUM"))
        _PS_BUFS = {"a": 2, "s": 2, "v": 2, "o": 2}

        def ps_tile(shape, tag):
            return psp.tile(shape, F32, name=tag, tag=tag, bufs=_PS_BUFS[tag])

        # ---- persistent SBUF (attention-critical loads emitted first) ----
        xqT_sb = one.tile([128, 2, RPC], F32R)
        nc.sync.dma_start(xqT_sb[:], xqT.rearrange("(t p) n -> p t n", t=2))
        msk_s = one.tile([128, NJT_S], F32)
        nc.sync.dma_start(msk_s[:], mask_s[:])
        xT_sb = one.tile([128, 2, T], F32R)

        def emit_xT_loads():
            for q4 in range(4):
                nc.sync.dma_start(xT_sb[:, :, q4 * (T // 4):(q4 + 1) * (T // 4)],
                                  xT.rearrange("(t p) n -> p t n", t=2)[:, :, q4 * (T // 4):(q4 + 1) * (T // 4)])
        # bulk loads needed only from the cross-attention / LN phase onward
        encT_sb = one.tile([128, 2, S], F32R)
        W1_sb = one.tile([128, 2, D], F32R)
        W2_sb = one.tile([128, 2, D], F32R)
        xr_sb = one.tile([128, 3, D], F32)
        cv = one.tile([128, 9, D], F32)
        msk_c = one.tile([128, NJT_C], F32)
        bfc_sb = one.tile([128, NVT], F32)
        b1p_sb = one.tile([128, 2], F32)
        vbc_sb = one.tile([128, H, 2], F32)
        ident = one.tile([128, 128], F32)

        def emit_bulk_loads():
            nc.sync.dma_start(encT_sb[:], encT.rearrange("(t p) n -> p t n", t=2))
            nc.sync.dma_start(W1_sb[:], W1.rearrange("(t p) n -> p t n", t=2))
            nc.sync.dma_start(W2_sb[:], W2.rearrange("(t p) n -> p t n", t=2))
            nc.sync.dma_start(xr_sb[:], xr.rearrange("(m p) d -> p m d", m=3))
            nc.sync.dma_start(cv[:], cvec[:])
            nc.sync.dma_start(msk_c[:], mask_c[:])
            nc.sync.dma_start(bfc_sb[:], bfcp[:])
            nc.sync.dma_start(b1p_sb[:], b1p[:])
            nc.sync.dma_start(vbc_sb[:], vbc[:])
            make_identity(nc, ident[:])

        xn_t = one.tile([128, 3, D], F32)
        acc_att = one.tile([128, 3, D], F32)
        tres = one.tile([128, 3, D], F32)
        x1_sb = one.tile([128, 3, D], F32)
        x2_sb = one.tile([128, 3, D], F32)
        x3_sb = one.tile([128, 3, D], F32)
        x1T_sb = one.tile([128, 2, RPC], F32R)
        x2T_sb = one.tile([128, 2, RPC], F32R)
        x3T_sb = one.tile([128, 2, RPC], BF16)
        h1T_sb = one.tile([128, 2, RPC], F32R)

        # ---- attention block (both self and cross) ----
        def compute_v2(keys_sb, njt, Ph, msk, v2):
            nc.vector.tensor_copy(v2[:, 0:njt, D:D + 1], msk[:, 0:njt].unsqueeze(-1))
            for jt in range(njt):
                v_ps = ps_tile([128, D], "v")
                for k in range(2):
                    nc.tensor.matmul(v_ps[:], keys_sb[:, k, jt * 128:(jt + 1) * 128],
                                     Ph[:, k, :], start=(k == 0), stop=(k == 1))
                nc.vector.tensor_scalar_mul(v2[:, jt, 0:D], v_ps[:], msk[:, jt:jt + 1])

        def attention(keys_sb, njt, q_sb, M_dram, P_dram, msk, acc, after_h0_dma=None,
                      a_bias=None):
            for h in range(H):
                Mh = hwp.tile([128, 2, D], F32R, tag="Mh")
                nc.sync.dma_start(
                    Mh[:], M_dram[:, h * D:(h + 1) * D].rearrange("(t p) n -> p t n", t=2))
                Ph = hwp.tile([128, 2, D], F32R, tag="Ph")
                nc.sync.dma_start(
                    Ph[:], P_dram[:, h * D:(h + 1) * D].rearrange("(t p) n -> p t n", t=2))
                if h == 0 and after_h0_dma is not None:
                    after_h0_dma()

                # A^T = M_h^T @ q^T : [256, RPC]
                a_sb = apool.tile([128, 2, RPC], F32R, tag="a")
                for dh in range(2):
                    a_ps = ps_tile([128, RPC], "a")
                    for k in range(2):
                        nc.tensor.matmul(a_ps[:], Mh[:, k, dh * 128:(dh + 1) * 128],
                                         q_sb[:, k, :], start=(k == 0), stop=(k == 1))
                    if a_bias is None:
                        nc.scalar.activation(a_sb[:, dh, :], a_ps[:], AF.Copy)
                    else:
                        nc.scalar.activation(a_sb[:, dh, :], a_ps[:], AF.Identity,
                                             bias=a_bias[:, h, dh:dh + 1])

                # S^T tiles + exp -> PT (bf16)
                pt = ptp.tile([128, NJT_S, RPC], BF16, tag="pt")
                for jt in range(njt):
                    s_ps = ps_tile([128, RPC], "s")
                    for k in range(2):
                        nc.tensor.matmul(s_ps[:], keys_sb[:, k, jt * 128:(jt + 1) * 128],
                                         a_sb[:, k, :], start=(k == 0), stop=(k == 1))
                    nc.scalar.activation(pt[:, jt, :], s_ps[:], AF.Exp)

                # V2' = [m ⊙ (keys @ P_h) | m]  (bf16)
                v2 = v2p.tile([128, NJT_S, D + 1], BF16, tag="v2")
                compute_v2(keys_sb, njt, Ph, msk, v2)

                # O = P^T.T @ V2' ; normalize rows by the mask-column sum
                for m in range(3):
                    o_ps = ps_tile([128, D + 1], "o")
                    for jt in range(njt):
                        nc.tensor.matmul(o_ps[:], pt[:, jt, m * 128:(m + 1) * 128],
                                         v2[:, jt, :], start=(jt == 0), stop=(jt == njt - 1))
                    r = sml.tile([128, 1], F32, tag="r")
                    nc.vector.reciprocal(r[:], o_ps[:, D:D + 1])
                    if h == 0:
                        nc.scalar.activation(acc[:, m, :], o_ps[:, 0:D], AF.Identity, scale=r[:])
                    else:
                        tmp = sml.tile([128, D], F32, tag="otmp")
                        nc.scalar.activation(tmp[:], o_ps[:, 0:D], AF.Identity, scale=r[:])
                        nc.vector.tensor_add(acc[:, m, :], acc[:, m, :], tmp[:])

        # ---- layernorm + optional transpose ----
        def layernorm(t_sb, g_idx, be_idx, out_sb, outT_sb):
            # t_sb: [128,3,D] fp32 pre-norm input (residual already added)
            mv = sml.tile([128, 3, 2], F32, tag="mv")
            for m in range(3):
                st = sml.tile([128, 6], F32, tag="st")
                nc.vector.bn_stats(st[:], t_sb[:, m, :])
                nc.vector.bn_aggr(mv[:, m, :], st[:])
            veps = sml.tile([128, 3, 1], F32, tag="veps")
            nc.vector.tensor_scalar_add(veps[:], mv[:, :, 1:2], EPS)
            stdv = sml.tile([128, 3, 1], F32, tag="stdv")
            nc.scalar.activation(stdv[:], veps[:], AF.Sqrt)
            rstd = sml.tile([128, 3, 1], F32, tag="rstd")
            nc.vector.reciprocal(rstd[:], stdv[:])
            nm = sml.tile([128, 3, 1], F32, tag="nm")
            nc.vector.tensor_scalar_mul(nm[:], mv[:, :, 0:1], -1.0)
            for m in range(3):
                # (t - mean) * rstd in one DVE op
                nc.vector.tensor_scalar(xn_t[:, m, :], t_sb[:, m, :], nm[:, m, 0:1],
                                        rstd[:, m, 0:1], mybir.AluOpType.add,
                                        mybir.AluOpType.mult)
            if outT_sb is not None:
                # gamma/beta are folded into the consumer weights on the host,
                # so the feature-major copy transposes the pre-affine values
                transpose_to(xn_t, outT_sb)
            for m in range(3):
                xn2_ = sml.tile([128, D], F32, tag="xn")
                nc.vector.tensor_mul(xn2_[:], xn_t[:, m, :], cv[:, g_idx, :])
                nc.vector.tensor_add(out_sb[:, m, :], xn2_[:], cv[:, be_idx, :])

        def transpose_to(src_sb, dstT_sb):
            # src [128,3,D] fp32 row-major -> dstT [128,2,RPC] feature-major
            for kk in range(2):
                for m in range(3):
                    t_ps = ps_tile([128, 128], "s" if (m * 2 + kk) % 2 == 0 else "v")
                    nc.tensor.transpose(t_ps[:], src_sb[:, m, kk * 128:(kk + 1) * 128], ident[:])
                    nc.scalar.activation(dstT_sb[:, kk, m * 128:(m + 1) * 128], t_ps[:], AF.Copy)

        # ================= forward =================
        # self-attention (keys = x of own batch, queries = own rows)
        attention(xT_sb, NJT_S, xqT_sb, Ms, Ps, msk_s, acc_att,
                  after_h0_dma=emit_xT_loads)
        emit_bulk_loads()
        for m in range(3):
            nc.vector.tensor_add(tres[:, m, :], acc_att[:, m, :], xr_sb[:, m, :])
        layernorm(tres, CVEC_G1, CVEC_BE1, x1_sb, x1T_sb)
        x1bo = one.tile([128, 3, D], F32)
        for m in range(3):
            nc.vector.tensor_add(x1bo[:, m, :], x1_sb[:, m, :], cv[:, CVEC_BOC, :])

        # cross-attention (keys = enc_output, queries = x1)
        attention(encT_sb, NJT_C, x1T_sb, Mc, Pc, msk_c, acc_att, a_bias=vbc_sb)
        for m in range(3):
            nc.vector.tensor_add(tres[:, m, :], acc_att[:, m, :], x1bo[:, m, :])
        layernorm(tres, CVEC_G2, CVEC_BE2, x2_sb, x2T_sb)
        x2b2 = one.tile([128, 3, D], F32)
        for m in range(3):
            nc.vector.tensor_add(x2b2[:, m, :], x2_sb[:, m, :], cv[:, CVEC_B2, :])

        # FFN: h1^T = gelu(W1^T @ x2^T + b1) computed feature-major (b1 is a
        # legal per-partition ACT bias there), so no transpose is needed.
        for kk in range(2):
            f_ps = ps_tile([128, RPC], "s")
            for k in range(2):
                nc.tensor.matmul(f_ps[:], W1_sb[:, k, kk * 128:(kk + 1) * 128],
                                 x2T_sb[:, k, :], start=(k == 0), stop=(k == 1))
            nc.scalar.activation(h1T_sb[:, kk, :], f_ps[:], AF.Gelu,
                                 bias=b1p_sb[:, kk:kk + 1])
        for m in range(3):
            f2_ps = ps_tile([128, D], "v")
            for k in range(2):
                nc.tensor.matmul(f2_ps[:], h1T_sb[:, k, m * 128:(m + 1) * 128],
                                 W2_sb[:, k, :], start=(k == 0), stop=(k == 1))
            nc.vector.tensor_add(tres[:, m, :], f2_ps[:], x2b2[:, m, :])
        layernorm(tres, CVEC_G3, CVEC_BE3, x3_sb, x3T_sb)

        nc.sync.dma_start(x3o.rearrange("(m p) d -> p m d", m=3), x3_sb[:])

        # ---- final vocab projection: logits^T = Wfc^T-tiles @ x3^T ----
        nblk = (V + WFC_BLK - 1) // WFC_BLK
        gv = 0
        for blk in range(nblk):
            c0 = blk * WFC_BLK
            w = min(WFC_BLK, V - c0)
            wt = wfcp.tile([128, 2, WFC_BLK], BF16, tag="wfc")
            nc.sync.dma_start(wt[:, :, 0:w],
                              Wfc.rearrange("(t p) n -> p t n", t=2)[:, :, c0:c0 + w])
            nvt_blk = (w + 127) // 128
            # group output tiles for batched DMA-out
            g = 0
            while g < nvt_blk:
                gn = min(VGRP, nvt_blk - g)
                lg = lgp.tile([128, VGRP, RPC], BF16, tag="lg")
                rows_full = 0
                for i in range(gn):
                    vt = g + i
                    pcount = min(128, w - vt * 128)
                    fc_ps = ps_tile([128, RPC], "s" if (gv + i) % 2 == 0 else "v")
                    for k in range(2):
                        nc.tensor.matmul(fc_ps[0:pcount, :],
                                         wt[:, k, vt * 128:vt * 128 + pcount],
                                         x3T_sb[:, k, :], start=(k == 0), stop=(k == 1))
                    bcol = bfc_sb[0:pcount, gv + i:gv + i + 1]
                    if (gv + i) % 2 == 0:
                        nc.scalar.activation(lg[0:pcount, i, :], fc_ps[0:pcount, :],
                                             AF.Identity, bias=bcol)
                    else:
                        nc.vector.tensor_scalar_add(lg[0:pcount, i, :], fc_ps[0:pcount, :],
                                                    bcol)
                    rows_full += pcount
                r0 = c0 + g * 128
                if rows_full == gn * 128:
                    dma_eng = nc.sync if (gv // VGRP) % 2 == 0 else nc.gpsimd
                    dma_eng.dma_start(
                        logitsT[r0:r0 + rows_full, :].rearrange("(g p) r -> p g r", g=gn),
                        lg[:, 0:gn, :])
                else:
                    # tail group: full tiles first, then the partial one
                    nfull = (rows_full) // 128
                    if nfull:
                        nc.sync.dma_start(
                            logitsT[r0:r0 + nfull * 128, :].rearrange("(g p) r -> p g r", g=nfull),
                            lg[:, 0:nfull, :])
                    prem = rows_full - nfull * 128
                    if prem:
                        nc.sync.dma_start(
                            logitsT[r0 + nfull * 128:r0 + rows_full, :],
                            lg[0:prem, nfull, :])
                g += gn
                gv += gn

    nc.compile()
    return nc


def _get_nc():
    if "nc" not in _CACHE:
        _CACHE["nc"] = _build_nc()
    return _CACHE["nc"]


def _host_prep(inputs):
    """Build the 8 per-core input maps from the full problem inputs."""
    f32 = np.float32
    src = np.asarray(inputs["src_input_id"])
    t1 = np.asarray(inputs["tgt_input_id_t1"])
    t2 = np.asarray(inputs["tgt_input_id_t2"])
    enc = np.asarray(inputs["enc_output"], dtype=f32)
    emb = np.asarray(inputs["token_emb"], dtype=f32)

    cls_col = np.full((B, 1), CLS, dtype=t1.dtype)
    tgt = np.concatenate([cls_col, t1, cls_col, t2], axis=1)        # [B, T]
    tgt_pad = (tgt == 0)
    src_pad = (src == 0)
    tgt_ids = np.where((tgt == MASK_TOK) | (tgt == 0), MASK_TOK, tgt).astype(np.int64)

    pe = _pos_encoding(T, D)
    x = (2.0 * emb[tgt_ids] + pe[None, :, :]).astype(f32)           # [B, T, D]

    # fold per-head weight products (weight-only algebra, done once per call)
    def head_products(Wq, Wk, Wv, Wo, scale):
        Wq = np.asarray(Wq, f32).reshape(D, H, DH)
        Wk = np.asarray(Wk, f32).reshape(D, H, DH)
        Wv = np.asarray(Wv, f32).reshape(D, H, DH)
        Wo = np.asarray(Wo, f32).reshape(H, DH, D)
        M = np.einsum("dhe,fhe->hdf", Wq * scale, Wk, optimize=True)  # [H, D, D]
        P = np.einsum("dhe,hef->hdf", Wv, Wo, optimize=True)          # [H, D, D]
        Mcat = np.ascontiguousarray(M.transpose(1, 0, 2).reshape(D, H * D), dtype=f32)
        Pcat = np.ascontiguousarray(P.transpose(1, 0, 2).reshape(D, H * D), dtype=f32)
        return Mcat, Pcat

    Ms_cat, Ps_cat = head_products(inputs["Wq_s"], inputs["Wk_s"], inputs["Wv_s"],
                                   inputs["Wo_s"], ATT_SCALE)
    Mc_cat, Pc_cat = head_products(inputs["Wq_c"], inputs["Wk_c"], inputs["Wv_c"],
                                   inputs["Wo_c"], ATT_SCALE)

    # fold each layernorm's affine into its downstream consumer:
    #   x1 = xn1*g1 + be1 feeds the cross-attn A-projection (Mc) only,
    #   x2 = xn2*g2 + be2 feeds W1 only, x3 = xn3*g3 + be3 feeds Wfc only.
    g1 = np.asarray(inputs["g1"], f32); be1 = np.asarray(inputs["be1"], f32)
    g2 = np.asarray(inputs["g2"], f32); be2 = np.asarray(inputs["be2"], f32)
    g3 = np.asarray(inputs["g3"], f32); be3 = np.asarray(inputs["be3"], f32)
    vbc_vec = be1 @ Mc_cat                          # [H*D] bias on A^T
    Mc_cat = np.ascontiguousarray(Mc_cat * g1[:, None])
    vbc = np.ascontiguousarray(vbc_vec.reshape(H, 2, 128).transpose(2, 0, 1))

    bo_s = np.asarray(inputs["bo_s"], f32)
    bo_c = np.asarray(inputs["bo_c"], f32)
    cvec = np.zeros((128, 9, D), dtype=f32)
    for i, name in enumerate([None, "g1", "be1", "g2", "be2", "g3", "be3", "b1", "b2"]):
        vecv = bo_c if name is None else np.asarray(inputs[name], f32)
        cvec[:, i, :] = vecv[None, :]

    W1 = np.asarray(inputs["W1"], f32)
    W2 = np.asarray(inputs["W2"], f32)
    b1_fold = np.asarray(inputs["b1"], f32) + be2 @ W1
    W1 = np.ascontiguousarray(W1 * g2[:, None])
    Wfc = np.asarray(inputs["Wfc"], f32)
    import ml_dtypes
    bfc = np.asarray(inputs["bfc"], f32) + be3 @ Wfc
    Wfc_bf = np.ascontiguousarray(Wfc * g3[:, None]).astype(ml_dtypes.bfloat16)
    bfc_pad = np.zeros(NVT * 128, dtype=f32)
    bfc_pad[:V] = bfc
    bfcp = np.ascontiguousarray(bfc_pad.reshape(NVT, 128).T)        # [128, NVT]

    in_maps = []
    for c in range(8):
        b = c // 4
        r0 = (c % 4) * RPC
        xb = x[b]                                                   # [T, D]
        xTb = np.ascontiguousarray(xb.T)                            # [D, T]
        keep_s = (~tgt_pad[b]).astype(f32)                          # [T]
        keep_c = (~src_pad[b]).astype(f32)                          # [S]
        in_maps.append({
            "xT": xTb,
            "xqT": np.ascontiguousarray(xb[r0:r0 + RPC].T),
            "encT": np.ascontiguousarray(enc[b].T),
            "Ms": Ms_cat, "Ps": Ps_cat, "Mc": Mc_cat, "Pc": Pc_cat,
            "W1": W1, "W2": W2, "Wfc": Wfc_bf,
            "xr": np.ascontiguousarray(xb[r0:r0 + RPC] + bo_s[None, :]),
            "cvec": cvec,
            "mask_s": np.ascontiguousarray(keep_s.reshape(NJT_S, 128).T),
            "mask_c": np.ascontiguousarray(keep_c.reshape(NJT_C, 128).T),
            "bfcp": bfcp,
            "b1p": np.ascontiguousarray(b1_fold.reshape(2, 128).T),
            "vbc": vbc,
        })
    return in_maps


def _assemble(results):
    logits = np.empty((B, T, V), dtype=np.float32)
    for c in range(8):
        b = c // 4
        r0 = (c % 4) * RPC
        logits[b, r0:r0 + RPC, :] = results[c]["logitsT"].T.astype(np.float32)
    x0 = np.stack([results[0]["x3o"][0], results[4]["x3o"][0]])
    return logits[:, 1:, :], x0


def kernel(**inputs):
    from concourse.bass_utils import run_bass_kernel_spmd
    in_maps = _host_prep(inputs)
    nc = _get_nc()
    res = run_bass_kernel_spmd(nc, in_maps, core_ids=list(range(8)))
    return _assemble(res.results)
